# revision 10
# baseline (speedup 1.0000x reference)
"""CodeGen attention (B=2, S=2048, E=4096, H=16, HD=256) on 8 trn2 NeuronCores.

Sharding: data parallel over batch (2) x tensor parallel over heads (4 groups
of 4 heads) = 8 cores. Each core projects its 4 heads' q/k/v (W columns
pre-gathered on host), applies rotary embedding, and runs causal attention.

All matmuls run in bf16 (fp32 PSUM accumulate) with explicit LDWEIGHTS
instructions and elided per-matmul self-loads (InstMatmult.ldweights=False):
bf16 weight loads use the fast-weight-load path (~40ns) and hide completely
in the inter-matmul gaps, so N=512 matmuls stream back-to-back at the
~216ns array floor (vs ~272ns for self-loading f32r).

Device dataflow per core (two token halves of 1024):
  K,Q projection (feature-major): W f-tile chunks stationary, X^T moving;
    PSUM (128 f x 1024 s) accumulated over 32 contraction chunks; RoPE fused
    into the PSUM->SBUF eviction on DVE (features pre-deinterleaved per head
    to [even dims; odd dims]); spilled to DRAM as bf16.
  V projection (token-major "natural" layout): X^T chunk token-slices
    stationary, W_v natural moving; per feat-half pass, 8 token-tile PSUM
    accumulators (1 bank each) filled chunk-major so X^T tiles free
    progressively (hides the second half's X reload); RoPE eviction writes a
    resident SBUF tile (k, d) -- no PE transposes needed in attention.
  Attention: per head, K^T loaded to SBUF; per 512-query block accumulate
    over causal k-tiles: P^T = K Q^T in PSUM -> exp (ScalarE, 1/16 scale
    folded) -> masked on diagonal tiles -> A V from the resident natural-V
    tile and a ones-row matmul for the softmax denominator -> reciprocal,
    normalize, DMA out (d x q).

Host reassembles the full (2, 2048, 4096) f32 output from the per-core
(1024 features x 2048 tokens) transposed shards.
"""

import numpy as np
import ml_dtypes

import concourse.bass as bass
import concourse.tile as tile
from concourse import bacc, mybir
from concourse.bass_utils import run_bass_kernel_spmd

F32 = mybir.dt.float32
BF16 = mybir.dt.bfloat16
MULT = mybir.AluOpType.mult
ADD = mybir.AluOpType.add
SUB = mybir.AluOpType.subtract

P = 128
B, S, E, H, HD = 2, 2048, 4096, 16, 256
NHC = 4            # heads per core
KK = E // P        # 32 contraction chunks
KH = KK // 2       # chunks per X^T sub-tile
SH = S // 2        # tokens per projection half
NTT = SH // P      # 8 token-tiles per half
QB = 512           # query block in attention
NQB = S // QB
KT_PER_QB = QB // P


def build_nc():
    nc = bacc.Bacc(None, target_bir_lowering=False, debug=False)

    hsT = nc.declare_dram_parameter("hsT", [E, S], BF16, isOutput=False)
    # f-tiles 0..7 = K (head-major, [even;odd] per head), 8..15 = Q
    wqk = nc.declare_dram_parameter("wqk", [16, P, KK, P], BF16, isOutput=False)
    # V natural weights: [fh, contract-part, chunk, 512 cols]
    wvn = nc.declare_dram_parameter("wvn", [2, P, KK, 512], BF16, isOutput=False)
    sinT = nc.declare_dram_parameter("sinT", [P, S], F32, isOutput=False)
    cosT = nc.declare_dram_parameter("cosT", [P, S], F32, isOutput=False)
    sinN = nc.declare_dram_parameter("sinN", [S, P], F32, isOutput=False)
    cosN = nc.declare_dram_parameter("cosN", [S, P], F32, isOutput=False)
    masks = nc.declare_dram_parameter("masks", [KT_PER_QB, P, QB], BF16, isOutput=False)
    out = nc.declare_dram_parameter("out", [2 * NHC, P, S], F32, isOutput=True)

    qkT = nc.dram_tensor("qkT", [16, P, S], BF16)

    def ldmm(w_ap, mms):
        """Explicit LDWEIGHTS + self-load-elided matmuls sharing it."""
        nc.tensor.ldweights(w_ap)
        for out_ap, rhs_ap, st, sp in mms:
            m = nc.tensor.matmul(out_ap, w_ap, rhs_ap, start=st, stop=sp)
            m.ins.ldweights = False

    with tile.TileContext(nc) as tc:
        with (
            tc.tile_pool(name="vn", bufs=1) as vn_pool,
            tc.tile_pool(name="cst", bufs=1) as cst_pool,
        ):
            # resident natural V: [token-part, token-tile, head, d(even|odd)]
            vn_all = vn_pool.tile([P, 2 * NTT, NHC, HD], BF16)
            ones_f = cst_pool.tile([P, P], F32)
            nc.vector.memset(ones_f[:], 1.0)
            ones = cst_pool.tile([P, P], BF16)
            nc.vector.tensor_copy(out=ones[:], in_=ones_f[:])
            mask_t = cst_pool.tile([P, KT_PER_QB, QB], BF16)

            # ---------------- phase 1: projection + RoPE ----------------
            with (
                tc.tile_pool(name="xt", bufs=1) as xt_pool,
                tc.tile_pool(name="wst", bufs=3) as w_pool,
                tc.tile_pool(name="wv", bufs=3) as wv_pool,
                tc.tile_pool(name="tab", bufs=1) as tab_pool,
                tc.tile_pool(name="ntab", bufs=2) as ntab_pool,
                tc.tile_pool(name="rop", bufs=2) as rop_pool,
            ):
                xt_h = [
                    xt_pool.tile([P, KH, SH], BF16, name=f"xt{kh}")
                    for kh in range(2)
                ]
                w_tiles = {}

                def w_load(th, f):
                    # 4 pieces so the transfer spreads across DMA queues
                    w = w_pool.tile([P, KK, P], BF16, tag="w")
                    for q in range(4):
                        nc.sync.dma_start(
                            out=w[:, 8 * q:8 * (q + 1), :],
                            in_=wqk[f, :, 8 * q:8 * (q + 1), :],
                        )
                    w_tiles[(th, f)] = w

                for th in range(2):
                    s0 = th * SH
                    if th == 0:
                        w_load(0, 0)
                        w_load(0, 1)
                    # X^T pieces chunk-major so early chunks land first
                    for kk in range(KK):
                        for pc in range(2):
                            c = slice(pc * 512, (pc + 1) * 512)
                            nc.sync.dma_start(
                                out=xt_h[kk // KH][:, kk % KH, c],
                                in_=hsT[kk * P:(kk + 1) * P,
                                        s0 + pc * 512:s0 + (pc + 1) * 512],
                            )
                    cs = tab_pool.tile([P, SH], F32, tag="cs")
                    sn = tab_pool.tile([P, SH], F32, tag="sn")
                    for pc in range(2):
                        c = slice(pc * 512, (pc + 1) * 512)
                        cg = slice(s0 + pc * 512, s0 + (pc + 1) * 512)
                        nc.sync.dma_start(out=cs[:, c], in_=cosT[:, cg])
                        nc.sync.dma_start(out=sn[:, c], in_=sinT[:, cg])
                    # V-weight tiles for this half: feat-half 0 loads during
                    # the QK section; feat-half 1 loads during the fh0 pass
                    wv_t = {}

                    def wv_load(fh, kh):
                        wv = wv_pool.tile([P, KH, 512], BF16, tag="wv")
                        for q in range(4):
                            ck = slice(kh * KH + 4 * q, kh * KH + 4 * (q + 1))
                            nc.sync.dma_start(
                                out=wv[:, 4 * q:4 * (q + 1), :],
                                in_=wvn[fh, :, ck, :],
                            )
                        wv_t[(fh, kh)] = wv

                    wv_load(0, 0)
                    wv_load(0, 1)

                    # K pairs (f 0..7) then Q pairs (f 8..15), head-ordered
                    with tc.tile_pool(name="pjp", bufs=4, space="PSUM") as pjp:
                        halves = (slice(0, 512), slice(512, 1024))
                        for pair in range(8):
                            fe, fo = 2 * pair, 2 * pair + 1
                            # staggered prefetch: one f-tile at pair start,
                            # one after fe's contraction (bufs=3 suffices)
                            nxt = [
                                (th, f) if f < 16 else (th + 1, f - 16)
                                for f in (2 * pair + 2, 2 * pair + 3)
                            ]
                            nxt = [(t, f) for t, f in nxt if t < 2]
                            if nxt:
                                w_load(*nxt[0])
                            ps_e = pjp.tile([P, SH], F32, tag="pj")
                            ps_o = pjp.tile([P, SH], F32, tag="pj")
                            if pair == 0:
                                # chunk-major across the pair: stream at the
                                # X-arrival rate during the initial load
                                for kk in range(KK):
                                    kh, k = kk // KH, kk % KH
                                    st, sp = (kk == 0), (kk == KK - 1)
                                    for f, ps in ((fe, ps_e), (fo, ps_o)):
                                        ldmm(w_tiles[(th, f)][:, kk, :], [
                                            (ps[:, c], xt_h[kh][:, k, c], st, sp)
                                            for c in halves
                                        ])
                                if len(nxt) > 1:
                                    w_load(*nxt[1])
                            else:
                                for fi, (f, ps) in enumerate(((fe, ps_e), (fo, ps_o))):
                                    for kk in range(KK):
                                        kh, k = kk // KH, kk % KH
                                        st, sp = (kk == 0), (kk == KK - 1)
                                        ldmm(w_tiles[(th, f)][:, kk, :], [
                                            (ps[:, c], xt_h[kh][:, k, c], st, sp)
                                            for c in halves
                                        ])
                                    if fi == 0 and len(nxt) > 1:
                                        w_load(*nxt[1])
                            w_tiles.pop((th, fe), None)
                            w_tiles.pop((th, fo), None)
                            # RoPE eviction (DVE, f32 PSUM in, bf16 out)
                            for c in halves:
                                t1 = rop_pool.tile([P, 512], F32, tag="t1")
                                t3 = rop_pool.tile([P, 512], F32, tag="t3")
                                oe = rop_pool.tile([P, 512], BF16, tag="oe")
                                oo = rop_pool.tile([P, 512], BF16, tag="oo")
                                nc.vector.tensor_tensor(t1[:], ps_e[:, c], cs[:, c], MULT)
                                nc.vector.tensor_tensor(oo[:], ps_e[:, c], sn[:, c], MULT)
                                nc.vector.tensor_tensor(oe[:], ps_o[:, c], sn[:, c], MULT)
                                nc.vector.tensor_tensor(t3[:], ps_o[:, c], cs[:, c], MULT)
                                nc.vector.tensor_tensor(oe[:], t1[:], oe[:], SUB)
                                nc.vector.tensor_tensor(oo[:], t3[:], oo[:], ADD)
                                cg = slice(s0 + c.start, s0 + c.stop)
                                nc.sync.dma_start(out=qkT[fe, :, cg], in_=oe[:])
                                nc.sync.dma_start(out=qkT[fo, :, cg], in_=oo[:])

                    # ---- V natural (token-major), 2 feat-half passes ----
                    for fh in range(2):
                        if fh == 0:
                            # prefetch fh1's weights during the fh0 pass
                            wv_load(1, 0)
                        with tc.tile_pool(name="pvp", bufs=8, space="PSUM") as pvp:
                            psv = [
                                pvp.tile([P, 2, HD], F32, tag="pv", name=f"pv{t}")
                                for t in range(NTT)
                            ]
                            for kh in range(2):
                                wv = wv_t.pop((fh, kh))
                                for k in range(KH):
                                    kk = kh * KH + k
                                    st, sp = (kk == 0), (kk == KK - 1)
                                    for t in range(NTT):
                                        ldmm(
                                            xt_h[kh][:, k, t * P:(t + 1) * P],
                                            [(psv[t][:, :, :], wv[:, k, :], st, sp)],
                                        )
                                if fh == 0 and kh == 0:
                                    wv_load(1, 1)
                            # RoPE (natural): evens/odds along free dim
                            for t in range(NTT):
                                tok = slice(s0 + t * P, s0 + (t + 1) * P)
                                csR = ntab_pool.tile([P, 2, P], F32, tag="csR")
                                snR = ntab_pool.tile([P, 2, P], F32, tag="snR")
                                for r in range(2):
                                    nc.sync.dma_start(out=csR[:, r, :], in_=cosN[tok, :])
                                    nc.sync.dma_start(out=snR[:, r, :], in_=sinN[tok, :])
                                ps = psv[t]
                                pe = ps[:, :, 0:P]
                                po = ps[:, :, P:HD]
                                v1 = rop_pool.tile([P, 2, P], F32, tag="v1")
                                v2 = rop_pool.tile([P, 2, P], F32, tag="v2")
                                v3 = rop_pool.tile([P, 2, P], F32, tag="v3")
                                v4 = rop_pool.tile([P, 2, P], F32, tag="v4")
                                tt = th * NTT + t
                                hs = slice(2 * fh, 2 * fh + 2)
                                nc.vector.tensor_tensor(v1[:], pe, csR[:], MULT)
                                nc.vector.tensor_tensor(v2[:], po, snR[:], MULT)
                                nc.vector.tensor_tensor(
                                    vn_all[:, tt, hs, 0:P], v1[:], v2[:], SUB
                                )
                                nc.vector.tensor_tensor(v3[:], po, csR[:], MULT)
                                nc.vector.tensor_tensor(v4[:], pe, snR[:], MULT)
                                nc.vector.tensor_tensor(
                                    vn_all[:, tt, hs, P:HD], v3[:], v4[:], ADD
                                )

            # ---------------- phase 2: attention ----------------
            for ktl in range(KT_PER_QB):
                nc.sync.dma_start(out=mask_t[:, ktl, :], in_=masks[ktl])
            with (
                tc.tile_pool(name="att", bufs=2) as att_pool,
                tc.tile_pool(name="qt", bufs=5) as qt_pool,
                tc.tile_pool(name="ep", bufs=5) as ep_pool,
                tc.tile_pool(name="on", bufs=4) as on_pool,
                tc.tile_pool(name="atp", bufs=2, space="PSUM") as at_psum,
                tc.tile_pool(name="avp", bufs=6, space="PSUM") as av_psum,
            ):
                pending_norm = []

                def flush_norm():
                    while pending_norm:
                        pj, pq0, pav0, pav1, pden = pending_norm.pop(0)
                        rb = on_pool.tile([P, QB], F32, tag="rb")
                        # ~18 correct bits; softmax denominators are benign
                        nc.vector.reciprocal_approx_fast(rb[:], pden[:])
                        for dc, av in ((0, pav0), (1, pav1)):
                            o = on_pool.tile([P, QB], F32, tag="o")
                            nc.vector.tensor_tensor(o[:], av[:], rb[:], MULT)
                            nc.sync.dma_start(
                                out=out[2 * pj + dc, :, pq0:pq0 + QB], in_=o[:]
                            )

                for j in range(NHC):
                    kt = att_pool.tile([P, 2, S], BF16, tag="kt")
                    for dc in range(2):
                        for pc in range(4):
                            c = slice(pc * 512, (pc + 1) * 512)
                            nc.sync.dma_start(
                                out=kt[:, dc, c], in_=qkT[2 * j + dc, :, c]
                            )
                    qts = []
                    for qb in range(NQB):
                        q0 = qb * QB
                        qt = qt_pool.tile([P, 2, QB], BF16, tag="qt")
                        for dc in range(2):
                            nc.sync.dma_start(
                                out=qt[:, dc, :],
                                in_=qkT[8 + 2 * j + dc, :, q0:q0 + QB],
                            )
                        qts.append(qt)
                    for qb in range(NQB):
                        q0 = qb * QB
                        flush_norm()
                        qt = qts[qb]
                        av0 = av_psum.tile([P, QB], F32, tag="av")
                        av1 = av_psum.tile([P, QB], F32, tag="av")
                        den = av_psum.tile([P, QB], F32, tag="av")
                        nkt = KT_PER_QB * (qb + 1)

                        def emit_av(kti, ep, st, sp):
                            ldmm(vn_all[:, kti, j, 0:P],
                                 [(av0[:], ep[:], st, sp)])
                            ldmm(vn_all[:, kti, j, P:HD],
                                 [(av1[:], ep[:], st, sp)])
                            ldmm(ones[:], [(den[:], ep[:], st, sp)])

                        pend = []
                        lag = 2 if nkt >= 8 else 1
                        for kti in range(nkt):
                            pt = at_psum.tile([P, QB], F32, tag="pt")
                            ks = slice(kti * P, (kti + 1) * P)
                            ldmm(kt[:, 0, ks], [(pt[:], qt[:, 0, :], True, False)])
                            ldmm(kt[:, 1, ks], [(pt[:], qt[:, 1, :], False, True)])
                            ep = ep_pool.tile([P, QB], BF16, tag="ep")
                            nc.scalar.activation(
                                ep[:], pt[:], mybir.ActivationFunctionType.Exp,
                                scale=1.0 / 16.0,
                            )
                            if kti >= KT_PER_QB * qb:
                                nc.vector.tensor_tensor(
                                    ep[:], ep[:],
                                    mask_t[:, kti - KT_PER_QB * qb, :], MULT,
                                )
                            pend.append((kti, ep))
                            if len(pend) > lag:
                                pk, pe_ = pend.pop(0)
                                emit_av(pk, pe_, pk == 0, False)
                        while pend:
                            pk, pe_ = pend.pop(0)
                            emit_av(pk, pe_, pk == 0, pk == nkt - 1)
                        pending_norm.append((j, q0, av0, av1, den))
                flush_norm()

    nc.finalize()
    return nc


_DEINT = np.concatenate([np.arange(0, HD, 2), np.arange(1, HD, 2)])


def _prep_core_inputs(hidden_states, sinusoidal_pos, W_qkv):
    """Build the 8 per-core input dicts (b-major: core = b*4 + hg)."""
    sin = np.ascontiguousarray(sinusoidal_pos[:, :HD // 2])   # (S, 128)
    cos = np.ascontiguousarray(sinusoidal_pos[:, HD // 2:])
    sinT = np.ascontiguousarray(sin.T)                        # (128, S)
    cosT = np.ascontiguousarray(cos.T)

    masks = np.zeros((KT_PER_QB, P, QB), dtype=np.float32)
    k_rel = np.arange(P)[:, None]
    q_rel = np.arange(QB)[None, :]
    for ktl in range(KT_PER_QB):
        masks[ktl] = (k_rel + ktl * P <= q_rel).astype(np.float32)
    masks = masks.astype(ml_dtypes.bfloat16)

    hsT_b = [
        np.ascontiguousarray(hidden_states[b].T).astype(ml_dtypes.bfloat16)
        for b in range(B)
    ]

    wqk_hg, wvn_hg = [], []
    for hg in range(4):  # 4 head groups
        heads = np.arange(NHC * hg, NHC * hg + NHC)
        feat = (heads[:, None] * HD + _DEINT[None, :]).reshape(-1)  # (1024,)
        # K tiles then Q tiles, head-major
        cols_qk = np.concatenate([3 * feat + 1, 3 * feat + 0])
        wq = W_qkv[:, cols_qk]                                # (E, 2048)
        wqk_a = np.ascontiguousarray(
            wq.reshape(KK, P, 16, P).transpose(2, 1, 0, 3)
        ).astype(ml_dtypes.bfloat16)
        cols_v = 3 * feat + 2
        wv = W_qkv[:, cols_v]                                 # (E, 1024)
        wvn_a = np.ascontiguousarray(
            wv.reshape(KK, P, 2, 512).transpose(2, 1, 0, 3)
        ).astype(ml_dtypes.bfloat16)
        wqk_hg.append(wqk_a)
        wvn_hg.append(wvn_a)

    sinN = np.ascontiguousarray(sin)   # (S, 128) token-major
    cosN = np.ascontiguousarray(cos)

    in_maps = []
    for c in range(8):
        b, hg = divmod(c, 4)
        in_maps.append({
            "hsT": hsT_b[b],
            "wqk": wqk_hg[hg],
            "wvn": wvn_hg[hg],
            "sinT": sinT,
            "cosT": cosT,
            "sinN": sinN,
            "cosN": cosN,
            "masks": masks,
        })
    return in_maps


def _assemble(results):
    out = np.empty((B, S, E), dtype=np.float32)
    for c in range(8):
        b, hg = divmod(c, 4)
        heads = np.arange(NHC * hg, NHC * hg + NHC)
        feat = (heads[:, None] * HD + _DEINT[None, :]).reshape(-1)  # (1024,)
        core_out = results[c]["out"].reshape(2 * NHC * P, S)        # (1024, S)
        out[b][:, feat] = core_out.T
    return out


def _numpy_reference(hidden_states, sinusoidal_pos, attention_mask, W_qkv, b_qkv):
    """Exact fallback for off-spec inputs (nonzero bias / partial mask)."""
    b, s, _ = hidden_states.shape
    x = hidden_states.astype(np.float64)
    qkv = x @ W_qkv.astype(np.float64) + b_qkv.astype(np.float64)
    qkv = qkv.reshape(b, s, E, 3)
    q = qkv[..., 0].reshape(b, s, H, HD)
    k = qkv[..., 1].reshape(b, s, H, HD)
    v = qkv[..., 2].reshape(b, s, H, HD)
    sin, cos = np.split(sinusoidal_pos.astype(np.float64), 2, axis=-1)
    sin_pos = np.stack([sin, sin], axis=-1).reshape(s, HD)
    cos_pos = np.stack([cos, cos], axis=-1).reshape(s, HD)

    def rot(layer):
        rh = np.stack([-layer[..., 1::2], layer[..., ::2]], axis=-1)
        rh = rh.reshape(layer.shape)
        return layer * cos_pos[None, :, None, :] + rh * sin_pos[None, :, None, :]

    q, k, v = rot(q), rot(k), rot(v)
    causal = np.tril(np.ones((s, s), dtype=bool))[None, None]
    mask = np.logical_and(causal, attention_mask)
    logits = np.einsum("bqhd,bkhd->bhqk", q, k) / np.sqrt(HD)
    logits = np.where(mask, logits, -np.inf)
    logits -= logits.max(axis=-1, keepdims=True)
    w = np.exp(logits)
    w /= w.sum(axis=-1, keepdims=True)
    o = np.einsum("bhqk,bkhd->bqhd", w, v)
    return o.reshape(b, s, E).astype(np.float32)


_NC_CACHE = []


def kernel(hidden_states, sinusoidal_pos, attention_mask, W_qkv, b_qkv):
    hidden_states = np.asarray(hidden_states, dtype=np.float32)
    sinusoidal_pos = np.asarray(sinusoidal_pos, dtype=np.float32)
    attention_mask = np.asarray(attention_mask)
    W_qkv = np.asarray(W_qkv, dtype=np.float32)
    b_qkv = np.asarray(b_qkv, dtype=np.float32)

    if not bool(attention_mask.astype(bool).all()) or bool(np.any(b_qkv)):
        return _numpy_reference(
            hidden_states, sinusoidal_pos, attention_mask, W_qkv, b_qkv
        )

    if not _NC_CACHE:
        _NC_CACHE.append(build_nc())
    nc = _NC_CACHE[0]
    in_maps = _prep_core_inputs(hidden_states, sinusoidal_pos, W_qkv)
    res = run_bass_kernel_spmd(nc, in_maps, core_ids=list(range(8)))
    return _assemble(res.results)
